# revision 24
# baseline (speedup 1.0000x reference)
"""Trainium2 Bass kernel for nn_Attention_37641093382387.

Dense transformer attention block:
  qkv = x @ Wqkv; q,k + RoPE; causal softmax attention; out @ Wproj + bproj

Sharding: 8 cores = 2 batches x 4 head-groups (4 heads each).  Each core
computes its batch's partial output for its head group; host sums the 4
group partials per batch and adds the bias.

Design notes (per-core, all matmuls bf16 -> f32 PSUM):
  - host passes x^T (pre-transposed, bf16) so no on-chip transposes needed
  - qT,kT computed in [hd, T] layout (lhsT=W block, rhs=xT); v in [T, hd]
  - RoPE rotate-half done with two half-width DVE muls whose output
    partition window differs from the input window (cross-quadrant
    crossbar move; sin table pre-rolled by 64 partitions and sign-folded
    on the host) -- no PE permutation matmul, no extra PSUM bank
  - attention transposed: ST[s,t] = k_tile^T @ q -> exp on ACT (scale
    folded) -> PT bf16; softmax denominators via DVE accumulation of the
    PT tiles (bf16) + ONE ones-matmul per (head, block) -- removes the
    per-tile ones-matmul pass from the PE (~10% of PE work)
  - emission is software-pipelined across blocks: attention of block j
    and proj of block j-1 are round-robin interleaved with the QKV
    chains of block j+1, so the ACT/DVE-heavy attention stream hides
    under QKV matmuls and output DMA is spread across the kernel
  - warmup matmuls on the first-landed weight chunk run during the
    initial DMA wait so the PE reaches HAM 8/8 before real work arrives
"""

import os
import sys
from collections import deque

import numpy as np

for _p in ("/opt/trn_rl_repo",):
    if _p not in sys.path and os.path.isdir(_p):
        sys.path.insert(0, _p)

import ml_dtypes

bf16 = ml_dtypes.bfloat16

P = 128
T = 2048
D = 2048
HD = 128
NG = 4      # head groups
HPG = 4     # heads per group
B = 2
BK = 512    # t block
NB = T // BK          # 4 t-blocks
NKT = D // P          # 16 contraction chunks
NTT = T // P          # 16 t-tiles
SCALE = float(HD) ** -0.5
SEG = 4               # matmuls per qkv yield segment

_NC_CACHE = {}

_DONE = object()


def _build_nc():
    import concourse.mybir as mybir
    from concourse import bacc
    from concourse.tile import TileContext

    fp32 = mybir.dt.float32
    bf = mybir.dt.bfloat16
    Exp = mybir.ActivationFunctionType.Exp

    nc = bacc.Bacc("TRN2", target_bir_lowering=False, debug=False,
                   num_devices=B * NG)

    xt_d = nc.declare_dram_parameter("xt", [NB, P, NKT, BK], bf,
                                     isOutput=False)
    wqk_d = nc.declare_dram_parameter("wqk", [2 * HPG, P, NKT, HD], bf,
                                      isOutput=False)
    wv_d = nc.declare_dram_parameter("wv", [P, NKT, HPG * HD], bf,
                                     isOutput=False)
    wp_d = nc.declare_dram_parameter("wp", [P, HPG, D], bf, isOutput=False)
    cos_d = nc.declare_dram_parameter("cos", [HD, T], bf, isOutput=False)
    sinr_d = nc.declare_dram_parameter("sinr", [HD, T], bf, isOutput=False)
    tri_d = nc.declare_dram_parameter("tri", [P, P], bf, isOutput=False)
    ones_d = nc.declare_dram_parameter("ones", [P, P], bf, isOutput=False)
    out_d = nc.declare_dram_parameter("out", [T, D], bf, isOutput=True)

    with TileContext(nc) as tc, \
         tc.tile_pool(name="const", bufs=1) as constp, \
         tc.tile_pool(name="persist", bufs=1) as persistp, \
         tc.tile_pool(name="xt", bufs=2) as xtp, \
         tc.tile_pool(name="sb", bufs=1) as sbpool, \
         tc.tile_pool(name="ps", bufs=1, space="PSUM") as pspool:

        # Per-tag buffer counts inside two merged pools (fewer pools =>
        # shorter NEFF epilogue sem ladder).  A view pins the per-tag
        # bufs so call sites stay unchanged.
        class _PoolView:
            def __init__(self, pool, bufs):
                self._pool, self._bufs = pool, bufs

            def tile(self, shape, dtype, tag="", name=None, bufs=None):
                return self._pool.tile(
                    shape, dtype, tag=tag, name=name,
                    bufs=self._bufs if bufs is None else bufs)

        qp = _PoolView(sbpool, 2)
        otp = _PoolView(sbpool, 3)
        workp = _PoolView(sbpool, 3)
        accp = _PoolView(sbpool, 2)
        ypool = _PoolView(sbpool, 3)
        ptp = _PoolView(sbpool, 4)
        psmm = _PoolView(pspool, 2)
        psst = _PoolView(pspool, 3)
        psop = _PoolView(pspool, 1)
        pssum = _PoolView(pspool, 1)
        pspy = _PoolView(pspool, 1)

        # ---- constant loads, in first-consumption order ----
        # wqk chunks are 4kt (128KB) early / 8kt later; xt block-0 chunks
        # are 4kt (512KB).  Interleaved so e-chain 0's kt stream is fed
        # with minimal first-byte latency.
        xt_sb0 = xtp.tile([P, NKT, BK], bf, tag="xt", name="xt_sb0")
        wqk_sb = constp.tile([P, 2 * HPG, NKT, HD], bf)
        # block-0 xt in 2kt (256KB) granules so e-chain 0 stalls at
        # finer boundaries while the stream ramps
        for c in range(4):
            nc.sync.dma_start(wqk_sb[:, 0, 4 * c:4 * (c + 1), :],
                              wqk_d[0, :, 4 * c:4 * (c + 1), :])
            for h2 in range(2):
                s = 4 * c + 2 * h2
                nc.sync.dma_start(xt_sb0[:, s:s + 2, :],
                                  xt_d[0, :, s:s + 2, :])
        cos_sb = constp.tile([HD, T], bf)
        sinr_sb = constp.tile([HD, T], bf)

        def load_wqk_e(e):
            for c in range(2):
                nc.sync.dma_start(wqk_sb[:, e, 8 * c:8 * (c + 1), :],
                                  wqk_d[e, :, 8 * c:8 * (c + 1), :])

        load_wqk_e(4)
        nc.sync.dma_start(cos_sb[:], cos_d[:])
        nc.sync.dma_start(sinr_sb[:], sinr_d[:])
        load_wqk_e(1)
        load_wqk_e(5)
        wv_sb = constp.tile([P, NKT, HPG * HD], bf)
        for c in range(2):
            nc.sync.dma_start(wv_sb[:, 8 * c:8 * (c + 1), :],
                              wv_d[:, 8 * c:8 * (c + 1), :])
        tri_sb = constp.tile([P, P], bf)
        nc.sync.dma_start(tri_sb[:], tri_d[:])
        ones_sb = constp.tile([P, P], bf)
        nc.sync.dma_start(ones_sb[:], ones_d[:])
        for e in (2, 6, 3, 7):
            load_wqk_e(e)
        wp_sb = constp.tile([P, HPG, D], bf)
        for c in range(4):
            nc.sync.dma_start(wp_sb[:, c, :], wp_d[:, c, :])

        # ---- persistent tensors ----
        k_sb = persistp.tile([HD, HPG, T], bf)        # kT per head
        v_sb = persistp.tile([P, NTT, HPG * HD], bf)  # v  per t-tile
        xt_tiles = {0: xt_sb0}
        q_tiles = {}
        ot_tiles = {}

        # ---- warmup matmuls: keep PE busy (and HAM warming) from the
        # moment the engines boot, through the initial DMA wait.  The
        # operand tile is never written -- stale SBUF is fine, results
        # are garbage, and every real accumulation starts with
        # start=True.  They write the (otherwise idle until S1) "sum"
        # bank -- NOT "mm", where the buf rotation against an open
        # chain would deadlock the PE queue.
        warm_sb = constp.tile([P, 2 * SEG, HD], bf, name="warm_sb")
        nc.vector.memset(warm_sb[:], 0)

        def warm_gen(n):
            # alternate between the two S0-idle psum banks: single-bank
            # rotation costs ~640ns of WAR-release wait per matmul
            for c in range(n):
                pool, tag = ((pssum, "sum"), (pspy, "py"))[c % 2]
                pswu = pool.tile([P, BK], fp32, tag=tag, name="pswu")
                nc.tensor.matmul(pswu[:], warm_sb[:, c % 4, :],
                                 warm_sb[:, 0:4, :], start=True, stop=True)
                yield

        for _ in warm_gen(8):
            pass

        # ================= emission generators =================

        def rope_tail(e, raw, tsl):
            """RoPE for one e-tile; all-bf16 DVE, rotate-half via
            cross-quadrant half-width muls against the rolled sin."""
            t1 = workp.tile([P, BK], bf, tag="t1", name="t1")
            nc.vector.tensor_mul(t1[:], raw[:], cos_sb[:, tsl])
            t2 = workp.tile([P, BK], bf, tag="t2", name="t2")
            nc.vector.tensor_mul(t2[0:HD // 2, :], raw[HD // 2:, :],
                                 sinr_sb[HD // 2:, tsl])
            nc.vector.tensor_mul(t2[HD // 2:, :], raw[0:HD // 2, :],
                                 sinr_sb[0:HD // 2, tsl])
            e_, j_ = e
            if e_ < HPG:
                dst = q_tiles[j_][:, e_, :]
            else:
                dst = k_sb[:, e_ - HPG, tsl]
            nc.vector.tensor_add(dst, t1[:], t2[:])

        def qkv_gen(chains, prefetch=None):
            """Emit qkv chains (list of ('e', j, e) / ('v', j, tt)),
            yielding after every SEG matmuls.  Rope tails are deferred by
            one yield so the ACT copy drains off the critical path."""
            if prefetch is not None and prefetch < NB:
                xt_nxt = xtp.tile([P, NKT, BK], bf, tag="xt",
                                  name=f"xt_sb{prefetch}")
                xt_tiles[prefetch] = xt_nxt
                for c in range(4):
                    nc.sync.dma_start(xt_nxt[:, 4 * c:4 * (c + 1), :],
                                      xt_d[prefetch, :, 4 * c:4 * (c + 1), :])
            pending = []

            def flush():
                while pending:
                    rope_tail(*pending.pop(0))

            for kind, j, idx in chains:
                tsl = slice(j * BK, (j + 1) * BK)
                xt_sb = xt_tiles[j]
                if kind == 'e':
                    if idx < HPG and j not in q_tiles:
                        q_tiles[j] = qp.tile([HD, HPG, BK], bf, tag="qblk",
                                             name=f"q_sb{j}")
                    ps = psmm.tile([P, BK], fp32, tag="mm", name="ps_qk")
                    for kt in range(NKT):
                        nc.tensor.matmul(
                            ps[:], wqk_sb[:, idx, kt, :], xt_sb[:, kt, :],
                            start=(kt == 0), stop=(kt == NKT - 1),
                        )
                        if kt % SEG == SEG - 1 and kt != NKT - 1:
                            yield
                    raw = workp.tile([P, BK], bf, tag="raw", name="raw")
                    nc.scalar.copy(raw[:], ps[:])
                    pending.append(((idx, j), raw, tsl))
                    yield
                    flush()
                else:
                    ps = psmm.tile([P, BK], fp32, tag="mm", name="ps_v")
                    for kt in range(NKT):
                        nc.tensor.matmul(
                            ps[:], xt_sb[:, kt, idx * P:(idx + 1) * P],
                            wv_sb[:, kt, :],
                            start=(kt == 0), stop=(kt == NKT - 1),
                        )
                        if kt % SEG == SEG - 1 and kt != NKT - 1:
                            yield
                    nc.scalar.copy(v_sb[:, 4 * j + idx, :], ps[:])
                    yield
            flush()

        def attn_gen(j):
            """Attention for block j, yielding after each (ST, exp, tri,
            acc) tile-stage; PV matmuls run 2 tiles behind."""
            ni = 4 * j + 4
            ot_tiles[j] = otp.tile([HD, HPG, BK], bf, tag="otblk",
                                   name=f"ot_sb{j}")
            ot_sb = ot_tiles[j]
            for h in range(HPG):
                pso = psop.tile([HD, BK], fp32, tag="o", name="pso")
                acc = accp.tile([P, BK], bf, tag="acc", name="acc")
                fifo = deque()

                def pv_stage(i, t0, pt):
                    nc.tensor.matmul(
                        pso[:, t0:], v_sb[:, i, h * HD:(h + 1) * HD],
                        pt[:, t0:],
                        start=(i == 0), stop=(i == ni - 1),
                    )

                for i in range(ni):
                    r = i - 4 * j
                    t0 = P * max(r, 0)
                    pst = psst.tile([P, BK], fp32, tag="st", name="pst")
                    nc.tensor.matmul(
                        pst[:, t0:],
                        k_sb[:, h, i * P:(i + 1) * P],
                        q_tiles[j][:, h, t0:],
                        start=True, stop=True,
                    )
                    pt = ptp.tile([P, BK], bf, tag="pt", name="pt")
                    nc.scalar.activation(pt[:, t0:], pst[:, t0:], Exp,
                                         scale=SCALE)
                    if r >= 0:
                        nc.vector.tensor_mul(
                            pt[:, t0:t0 + P], pt[:, t0:t0 + P], tri_sb[:]
                        )
                    if i == 0:
                        nc.vector.tensor_copy(acc[:], pt[:])
                    else:
                        nc.vector.tensor_add(acc[:, t0:], acc[:, t0:],
                                             pt[:, t0:])
                    fifo.append((i, t0, pt))
                    if len(fifo) > 2:
                        pv_stage(*fifo.popleft())
                    yield
                while fifo:
                    pv_stage(*fifo.popleft())
                # denominators: one ones-matmul over the DVE-accumulated
                # PT sum, then normalize.
                pss = pssum.tile([P, BK], fp32, tag="sum", name="pss")
                nc.tensor.matmul(pss[:], ones_sb[:], acc[:],
                                 start=True, stop=True)
                recip = workp.tile([P, BK], fp32, tag="recip", name="recip")
                nc.vector.reciprocal_approx_fast(recip[:], pss[:])
                nc.vector.tensor_mul(ot_sb[:, h, :], pso[:], recip[:])

        def proj_gen(j, pool=None, tag="py", row_batch=True):
            """Proj for block j, yielding per output tile.  S4/S5 pass
            the (then idle) qkv chain pool for 2-deep psum rotation.
            row_batch=False issues per-tile output DMAs as soon as each
            copy lands -- used for the final block so the output drain
            starts early and the last transfer is small."""
            pool = pspy if pool is None else pool
            ot_sb = ot_tiles[j]
            for tt in range(BK // P):
                # one [128, D] row-tile per tt: 4KB-contiguous partition
                # rows -> a single efficient 512KB output DMA
                y = ypool.tile([P, D], bf, tag="y", name="y")
                for n in range(D // BK):
                    psy = pool.tile([P, BK], fp32, tag=tag, name="psy")
                    for h in range(HPG):
                        nc.tensor.matmul(
                            psy[:],
                            ot_sb[:, h, tt * P:(tt + 1) * P],
                            wp_sb[:, h, n * BK:(n + 1) * BK],
                            start=(h == 0), stop=(h == HPG - 1),
                        )
                    if (tt * (D // BK) + n) % 2 == 0:
                        nc.scalar.copy(y[:, n * BK:(n + 1) * BK], psy[:])
                    else:
                        nc.vector.tensor_copy(y[:, n * BK:(n + 1) * BK],
                                              psy[:])
                    if not row_batch:
                        nc.sync.dma_start(
                            out_d[(j * 4 + tt) * P:(j * 4 + tt + 1) * P,
                                  n * BK:(n + 1) * BK],
                            y[:, n * BK:(n + 1) * BK],
                        )
                    yield
                if row_batch:
                    nc.sync.dma_start(
                        out_d[(j * 4 + tt) * P:(j * 4 + tt + 1) * P, :],
                        y[:],
                    )

        def drive(*pairs):
            """Weighted round-robin over generators until exhausted."""
            active = list(pairs)
            while active:
                keep = []
                for g, w in active:
                    alive = True
                    for _ in range(w):
                        if next(g, _DONE) is _DONE:
                            alive = False
                            break
                    if alive:
                        keep.append((g, w))
                active = keep

        def chains_for(j, es):
            return [('e', j, e) for e in es]

        def vchains_for(j):
            return [('v', j, tt) for tt in range(4)]

        # ================= stage schedule =================
        # S0: first half of qkv(0) -- DMA-paced; warmup matmuls fill the
        # stalls while the first xt/wqk chunks stream in
        g0a = qkv_gen(chains_for(0, [0, 4]))
        drive((g0a, 1), (warm_gen(8), 1))
        g0b = qkv_gen(chains_for(0, [1, 5]) + vchains_for(0), prefetch=1)
        drive((g0b, 1), (warm_gen(4), 1))

        # S1: attn(0) x [rest of qkv(0) + qkv(1)]
        g_q1 = qkv_gen(chains_for(0, [2, 6, 3, 7])
                       + chains_for(1, [0, 4, 1, 5, 2, 6, 3, 7])
                       + vchains_for(1),
                       prefetch=2)
        drive((attn_gen(0), 1), (g_q1, 4))

        # S2: attn(1) x qkv(2) x proj(0)
        g_q2 = qkv_gen(chains_for(2, [0, 4, 1, 5, 2, 6, 3, 7])
                       + vchains_for(2),
                       prefetch=3)
        drive((attn_gen(1), 2), (g_q2, 3), (proj_gen(0), 1))

        # S3: attn(2) x qkv(3); proj(1) deferred to the ACT-bound S4
        g_q3 = qkv_gen(chains_for(3, [0, 4, 1, 5, 2, 6, 3, 7])
                       + vchains_for(3))
        drive((attn_gen(2), 1), (g_q3, 1))

        # S4: attn(3) x proj(1) x proj(2) -- attention weighted so it
        # finishes with proj runway left (lets the final norms drain
        # before S5's first tile needs ot(3))
        drive((attn_gen(3), 5), (proj_gen(1), 1),
              (proj_gen(2, pool=psmm, tag="mm"), 1))

        # S5: proj(3) -- per-tile output DMAs to shorten the final drain
        for _ in proj_gen(3, pool=psmm, tag="mm", row_batch=False):
            pass

    nc.compile()
    return nc


def _get_nc():
    if "nc" not in _NC_CACHE:
        _NC_CACHE["nc"] = _build_nc()
    return _NC_CACHE["nc"]


def _host_prep(x, Wqkv, Wproj, mask):
    """Build the 8 per-core input maps (host-side layout transforms)."""
    x = np.asarray(x, dtype=np.float32)
    Wqkv = np.asarray(Wqkv, dtype=np.float32)
    Wproj = np.asarray(Wproj, dtype=np.float32)
    mask = np.asarray(mask, dtype=np.float32)

    # RoPE tables (transposed layout [hd, T]); standard rotate-half RoPE
    # with base 10000.  Signs folded into sin; sinr additionally rolled
    # by 64 partitions so the DVE half-width muls read input and table
    # from the same partition window.
    inv_freq = 1.0 / (10000.0 ** (np.arange(0, HD, 2, dtype=np.float32) / HD))
    freqs = np.arange(T, dtype=np.float32)[:, None] * inv_freq[None, :]
    emb = np.concatenate([freqs, freqs], axis=-1)        # [T, 128]
    cosT = np.ascontiguousarray(np.cos(emb).T).astype(bf16)
    sinT_f = np.ascontiguousarray(np.sin(emb).T)
    sinT_f[:HD // 2] *= -1.0
    sinr = np.ascontiguousarray(np.roll(sinT_f, HD // 2, axis=0)).astype(bf16)

    # [s', t''] triangle for the diagonal 128x128 tile, from the real mask
    tri = (mask[:P, :P].T == 0.0).astype(bf16)
    ones = np.ones((P, P), dtype=bf16)

    in_maps = []
    for b in range(B):
        xT = np.ascontiguousarray(x[b].T).astype(bf16)      # [D, T]
        # [NB, P, NKT, BK]: per (j, partition) rows are 16KB contiguous
        xt_pre = np.ascontiguousarray(
            xT.reshape(NKT, P, NB, BK).transpose(2, 1, 0, 3))
        for g in range(NG):
            heads = list(range(HPG * g, HPG * (g + 1)))
            wq = [Wqkv[:, h * HD:(h + 1) * HD] for h in heads]
            wk = [Wqkv[:, D + h * HD:D + (h + 1) * HD] for h in heads]
            wvl = [Wqkv[:, 2 * D + h * HD:2 * D + (h + 1) * HD] for h in heads]
            # [8, P, NKT, HD]: per-(e,partition) rows 4KB contiguous
            wqk = np.ascontiguousarray(
                np.stack(wq + wk, axis=0).astype(bf16)
                .reshape(2 * HPG, NKT, P, HD).transpose(0, 2, 1, 3))
            wv = np.ascontiguousarray(
                np.concatenate(wvl, axis=1).astype(bf16)
                .reshape(NKT, P, HPG * HD).transpose(1, 0, 2))
            wp = np.ascontiguousarray(
                Wproj[HPG * HD * g:HPG * HD * (g + 1), :].astype(bf16)
                .reshape(HPG, P, D).transpose(1, 0, 2))
            in_maps.append({
                "xt": xt_pre, "wqk": wqk, "wv": wv, "wp": wp,
                "cos": cosT, "sinr": sinr, "tri": tri, "ones": ones,
            })
    return in_maps


def run(x, Wqkv, Wproj, bproj, mask, trace=False):
    """Run the SPMD kernel; returns (output, BassKernelResults)."""
    from concourse.bass_utils import run_bass_kernel_spmd

    nc = _get_nc()
    in_maps = _host_prep(x, Wqkv, Wproj, mask)
    res = run_bass_kernel_spmd(nc, in_maps, core_ids=list(range(B * NG)),
                               trace=trace)

    bproj = np.asarray(bproj, dtype=np.float32)
    out = np.zeros((B, T, D), dtype=np.float32)
    for b in range(B):
        acc = np.zeros((T, D), dtype=np.float32)
        for g in range(NG):
            acc += np.asarray(res.results[b * NG + g]["out"], dtype=np.float32)
        out[b] = acc + bproj[None, :]
    return out, res


def kernel(x, Wqkv, Wproj, bproj, mask):
    out, _ = run(x, Wqkv, Wproj, bproj, mask, trace=False)
    return out


# revision 26
# speedup vs baseline: 1.0039x; 1.0039x over previous
"""Trainium2 Bass kernel for nn_Attention_37641093382387.

Dense transformer attention block:
  qkv = x @ Wqkv; q,k + RoPE; causal softmax attention; out @ Wproj + bproj

Sharding: 8 cores = 2 batches x 4 head-groups (4 heads each).  Each core
computes its batch's partial output for its head group; host sums the 4
group partials per batch and adds the bias.

Design notes (per-core, all matmuls bf16 -> f32 PSUM):
  - host passes x^T (pre-transposed, bf16) so no on-chip transposes needed
  - qT,kT computed in [hd, T] layout (lhsT=W block, rhs=xT); v in [T, hd]
  - RoPE rotate-half done with two half-width DVE muls whose output
    partition window differs from the input window (cross-quadrant
    crossbar move; sin table pre-rolled by 64 partitions and sign-folded
    on the host) -- no PE permutation matmul, no extra PSUM bank
  - attention transposed: ST[s,t] = k_tile^T @ q -> exp on ACT (scale
    folded) -> PT bf16; softmax denominators via DVE accumulation of the
    PT tiles (bf16) + ONE ones-matmul per (head, block) -- removes the
    per-tile ones-matmul pass from the PE (~10% of PE work)
  - emission is software-pipelined across blocks: attention of block j
    and proj of block j-1 are round-robin interleaved with the QKV
    chains of block j+1, so the ACT/DVE-heavy attention stream hides
    under QKV matmuls and output DMA is spread across the kernel
  - warmup matmuls on the first-landed weight chunk run during the
    initial DMA wait so the PE reaches HAM 8/8 before real work arrives
"""

import os
import sys
from collections import deque

import numpy as np

for _p in ("/opt/trn_rl_repo",):
    if _p not in sys.path and os.path.isdir(_p):
        sys.path.insert(0, _p)

import ml_dtypes

bf16 = ml_dtypes.bfloat16

P = 128
T = 2048
D = 2048
HD = 128
NG = 4      # head groups
HPG = 4     # heads per group
B = 2
BK = 512    # t block
NB = T // BK          # 4 t-blocks
NKT = D // P          # 16 contraction chunks
NTT = T // P          # 16 t-tiles
SCALE = float(HD) ** -0.5
SEG = 4               # matmuls per qkv yield segment

_NC_CACHE = {}

_DONE = object()


def _build_nc():
    import concourse.mybir as mybir
    from concourse import bacc
    from concourse.tile import TileContext

    fp32 = mybir.dt.float32
    bf = mybir.dt.bfloat16
    Exp = mybir.ActivationFunctionType.Exp

    nc = bacc.Bacc("TRN2", target_bir_lowering=False, debug=False,
                   num_devices=B * NG)

    xt_d = nc.declare_dram_parameter("xt", [NB, P, NKT, BK], bf,
                                     isOutput=False)
    wqk_d = nc.declare_dram_parameter("wqk", [2 * HPG, P, NKT, HD], bf,
                                      isOutput=False)
    wv_d = nc.declare_dram_parameter("wv", [P, NKT, HPG * HD], bf,
                                     isOutput=False)
    wp_d = nc.declare_dram_parameter("wp", [P, HPG, D], bf, isOutput=False)
    cos_d = nc.declare_dram_parameter("cos", [HD, T], bf, isOutput=False)
    sinr_d = nc.declare_dram_parameter("sinr", [HD, T], bf, isOutput=False)
    tri_d = nc.declare_dram_parameter("tri", [P, P], bf, isOutput=False)
    ones_d = nc.declare_dram_parameter("ones", [P, P], bf, isOutput=False)
    out_d = nc.declare_dram_parameter("out", [T, D], bf, isOutput=True)

    with TileContext(nc) as tc, \
         tc.tile_pool(name="const", bufs=1) as constp, \
         tc.tile_pool(name="persist", bufs=1) as persistp, \
         tc.tile_pool(name="xt", bufs=2) as xtp, \
         tc.tile_pool(name="sb", bufs=1) as sbpool, \
         tc.tile_pool(name="ps", bufs=1, space="PSUM") as pspool:

        # Per-tag buffer counts inside two merged pools (fewer pools =>
        # shorter NEFF epilogue sem ladder).  A view pins the per-tag
        # bufs so call sites stay unchanged.
        class _PoolView:
            def __init__(self, pool, bufs):
                self._pool, self._bufs = pool, bufs

            def tile(self, shape, dtype, tag="", name=None, bufs=None):
                return self._pool.tile(
                    shape, dtype, tag=tag, name=name,
                    bufs=self._bufs if bufs is None else bufs)

        qp = _PoolView(sbpool, 2)
        otp = _PoolView(sbpool, 3)
        workp = _PoolView(sbpool, 3)
        accp = _PoolView(sbpool, 2)
        ypool = _PoolView(sbpool, 3)
        ptp = _PoolView(sbpool, 4)
        psmm = _PoolView(pspool, 2)
        psst = _PoolView(pspool, 3)
        psop = _PoolView(pspool, 1)
        pssum = _PoolView(pspool, 1)
        pspy = _PoolView(pspool, 1)

        # ---- constant loads, in first-consumption order ----
        # wqk chunks are 4kt (128KB) early / 8kt later; xt block-0 chunks
        # are 4kt (512KB).  Interleaved so e-chain 0's kt stream is fed
        # with minimal first-byte latency.
        xt_sb0 = xtp.tile([P, NKT, BK], bf, tag="xt", name="xt_sb0")
        wqk_sb = constp.tile([P, 2 * HPG, NKT, HD], bf)
        for c in range(4):
            nc.sync.dma_start(wqk_sb[:, 0, 4 * c:4 * (c + 1), :],
                              wqk_d[0, :, 4 * c:4 * (c + 1), :])
            nc.sync.dma_start(xt_sb0[:, 4 * c:4 * (c + 1), :],
                              xt_d[0, :, 4 * c:4 * (c + 1), :])
        cos_sb = constp.tile([HD, T], bf)
        sinr_sb = constp.tile([HD, T], bf)

        def load_wqk_e(e):
            for c in range(2):
                nc.sync.dma_start(wqk_sb[:, e, 8 * c:8 * (c + 1), :],
                                  wqk_d[e, :, 8 * c:8 * (c + 1), :])

        load_wqk_e(4)
        nc.sync.dma_start(cos_sb[:], cos_d[:])
        nc.sync.dma_start(sinr_sb[:], sinr_d[:])
        load_wqk_e(1)
        load_wqk_e(5)
        wv_sb = constp.tile([P, NKT, HPG * HD], bf)
        for c in range(2):
            nc.sync.dma_start(wv_sb[:, 8 * c:8 * (c + 1), :],
                              wv_d[:, 8 * c:8 * (c + 1), :])
        tri_sb = constp.tile([P, P], bf)
        nc.sync.dma_start(tri_sb[:], tri_d[:])
        ones_sb = constp.tile([P, P], bf)
        nc.sync.dma_start(ones_sb[:], ones_d[:])
        for e in (2, 6, 3, 7):
            load_wqk_e(e)
        wp_sb = constp.tile([P, HPG, D], bf)
        for c in range(4):
            nc.sync.dma_start(wp_sb[:, c, :], wp_d[:, c, :])

        # ---- persistent tensors ----
        k_sb = persistp.tile([HD, HPG, T], bf)        # kT per head
        v_sb = persistp.tile([P, NTT, HPG * HD], bf)  # v  per t-tile
        xt_tiles = {0: xt_sb0}
        q_tiles = {}
        ot_tiles = {}

        # ---- warmup matmuls: keep PE busy (and HAM warming) from the
        # moment the engines boot, through the initial DMA wait.  The
        # operand tile is never written -- stale SBUF is fine, results
        # are garbage, and every real accumulation starts with
        # start=True.  They write the (otherwise idle until S1) "sum"
        # bank -- NOT "mm", where the buf rotation against an open
        # chain would deadlock the PE queue.
        warm_sb = constp.tile([P, 2 * SEG, HD], bf, name="warm_sb")
        nc.vector.memset(warm_sb[:], 0)

        def warm_gen(n):
            # alternate between the two S0-idle psum banks: single-bank
            # rotation costs ~640ns of WAR-release wait per matmul
            for c in range(n):
                pool, tag = ((pssum, "sum"), (pspy, "py"))[c % 2]
                pswu = pool.tile([P, BK], fp32, tag=tag, name="pswu")
                nc.tensor.matmul(pswu[:], warm_sb[:, c % 4, :],
                                 warm_sb[:, 0:4, :], start=True, stop=True)
                yield

        for _ in warm_gen(8):
            pass

        # ================= emission generators =================

        def rope_tail(e, raw, tsl):
            """RoPE for one e-tile; all-bf16 DVE, rotate-half via
            cross-quadrant half-width muls against the rolled sin."""
            t1 = workp.tile([P, BK], bf, tag="t1", name="t1")
            nc.vector.tensor_mul(t1[:], raw[:], cos_sb[:, tsl])
            t2 = workp.tile([P, BK], bf, tag="t2", name="t2")
            nc.vector.tensor_mul(t2[0:HD // 2, :], raw[HD // 2:, :],
                                 sinr_sb[HD // 2:, tsl])
            nc.vector.tensor_mul(t2[HD // 2:, :], raw[0:HD // 2, :],
                                 sinr_sb[0:HD // 2, tsl])
            e_, j_ = e
            if e_ < HPG:
                dst = q_tiles[j_][:, e_, :]
            else:
                dst = k_sb[:, e_ - HPG, tsl]
            nc.vector.tensor_add(dst, t1[:], t2[:])

        def qkv_gen(chains, prefetch=None):
            """Emit qkv chains (list of ('e', j, e) / ('v', j, tt)),
            yielding after every SEG matmuls.  Rope tails are deferred by
            one yield so the ACT copy drains off the critical path."""
            if prefetch is not None and prefetch < NB:
                xt_nxt = xtp.tile([P, NKT, BK], bf, tag="xt",
                                  name=f"xt_sb{prefetch}")
                xt_tiles[prefetch] = xt_nxt
                for c in range(4):
                    nc.sync.dma_start(xt_nxt[:, 4 * c:4 * (c + 1), :],
                                      xt_d[prefetch, :, 4 * c:4 * (c + 1), :])
            pending = []

            def flush():
                while pending:
                    rope_tail(*pending.pop(0))

            for kind, j, idx in chains:
                tsl = slice(j * BK, (j + 1) * BK)
                xt_sb = xt_tiles[j]
                if kind == 'e':
                    if idx < HPG and j not in q_tiles:
                        q_tiles[j] = qp.tile([HD, HPG, BK], bf, tag="qblk",
                                             name=f"q_sb{j}")
                    ps = psmm.tile([P, BK], fp32, tag="mm", name="ps_qk")
                    for kt in range(NKT):
                        nc.tensor.matmul(
                            ps[:], wqk_sb[:, idx, kt, :], xt_sb[:, kt, :],
                            start=(kt == 0), stop=(kt == NKT - 1),
                        )
                        if kt % SEG == SEG - 1 and kt != NKT - 1:
                            yield
                    raw = workp.tile([P, BK], bf, tag="raw", name="raw")
                    nc.scalar.copy(raw[:], ps[:])
                    pending.append(((idx, j), raw, tsl))
                    yield
                    flush()
                else:
                    ps = psmm.tile([P, BK], fp32, tag="mm", name="ps_v")
                    for kt in range(NKT):
                        nc.tensor.matmul(
                            ps[:], xt_sb[:, kt, idx * P:(idx + 1) * P],
                            wv_sb[:, kt, :],
                            start=(kt == 0), stop=(kt == NKT - 1),
                        )
                        if kt % SEG == SEG - 1 and kt != NKT - 1:
                            yield
                    nc.scalar.copy(v_sb[:, 4 * j + idx, :], ps[:])
                    yield
            flush()

        def attn_gen(j):
            """Attention for block j, yielding after each (ST, exp, tri,
            acc) tile-stage; PV matmuls run 2 tiles behind."""
            ni = 4 * j + 4
            ot_tiles[j] = otp.tile([HD, HPG, BK], bf, tag="otblk",
                                   name=f"ot_sb{j}")
            ot_sb = ot_tiles[j]
            for h in range(HPG):
                pso = psop.tile([HD, BK], fp32, tag="o", name="pso")
                acc = accp.tile([P, BK], bf, tag="acc", name="acc")
                fifo = deque()

                def pv_stage(i, t0, pt):
                    nc.tensor.matmul(
                        pso[:, t0:], v_sb[:, i, h * HD:(h + 1) * HD],
                        pt[:, t0:],
                        start=(i == 0), stop=(i == ni - 1),
                    )

                for i in range(ni):
                    r = i - 4 * j
                    t0 = P * max(r, 0)
                    pst = psst.tile([P, BK], fp32, tag="st", name="pst")
                    nc.tensor.matmul(
                        pst[:, t0:],
                        k_sb[:, h, i * P:(i + 1) * P],
                        q_tiles[j][:, h, t0:],
                        start=True, stop=True,
                    )
                    pt = ptp.tile([P, BK], bf, tag="pt", name="pt")
                    nc.scalar.activation(pt[:, t0:], pst[:, t0:], Exp,
                                         scale=SCALE)
                    if r >= 0:
                        nc.vector.tensor_mul(
                            pt[:, t0:t0 + P], pt[:, t0:t0 + P], tri_sb[:]
                        )
                    if i == 0:
                        nc.vector.tensor_copy(acc[:], pt[:])
                    else:
                        nc.vector.tensor_add(acc[:, t0:], acc[:, t0:],
                                             pt[:, t0:])
                    fifo.append((i, t0, pt))
                    if len(fifo) > 2:
                        pv_stage(*fifo.popleft())
                    yield
                while fifo:
                    pv_stage(*fifo.popleft())
                # denominators: one ones-matmul over the DVE-accumulated
                # PT sum, then normalize.
                pss = pssum.tile([P, BK], fp32, tag="sum", name="pss")
                nc.tensor.matmul(pss[:], ones_sb[:], acc[:],
                                 start=True, stop=True)
                recip = workp.tile([P, BK], fp32, tag="recip", name="recip")
                nc.vector.reciprocal_approx_fast(recip[:], pss[:])
                nc.vector.tensor_mul(ot_sb[:, h, :], pso[:], recip[:])

        def proj_gen(j, pool=None, tag="py", row_batch=True):
            """Proj for block j, yielding per output tile.  S4/S5 pass
            the (then idle) qkv chain pool for 2-deep psum rotation.
            row_batch=False issues per-tile output DMAs as soon as each
            copy lands -- used for the final block so the output drain
            starts early and the last transfer is small."""
            pool = pspy if pool is None else pool
            ot_sb = ot_tiles[j]
            for tt in range(BK // P):
                # one [128, D] row-tile per tt: 4KB-contiguous partition
                # rows -> a single efficient 512KB output DMA
                y = ypool.tile([P, D], bf, tag="y", name="y")
                for n in range(D // BK):
                    psy = pool.tile([P, BK], fp32, tag=tag, name="psy")
                    for h in range(HPG):
                        nc.tensor.matmul(
                            psy[:],
                            ot_sb[:, h, tt * P:(tt + 1) * P],
                            wp_sb[:, h, n * BK:(n + 1) * BK],
                            start=(h == 0), stop=(h == HPG - 1),
                        )
                    if (tt * (D // BK) + n) % 2 == 0:
                        nc.scalar.copy(y[:, n * BK:(n + 1) * BK], psy[:])
                    else:
                        nc.vector.tensor_copy(y[:, n * BK:(n + 1) * BK],
                                              psy[:])
                    if not row_batch:
                        nc.sync.dma_start(
                            out_d[(j * 4 + tt) * P:(j * 4 + tt + 1) * P,
                                  n * BK:(n + 1) * BK],
                            y[:, n * BK:(n + 1) * BK],
                        )
                    yield
                if row_batch:
                    nc.sync.dma_start(
                        out_d[(j * 4 + tt) * P:(j * 4 + tt + 1) * P, :],
                        y[:],
                    )

        def drive(*pairs):
            """Weighted round-robin over generators until exhausted."""
            active = list(pairs)
            while active:
                keep = []
                for g, w in active:
                    alive = True
                    for _ in range(w):
                        if next(g, _DONE) is _DONE:
                            alive = False
                            break
                    if alive:
                        keep.append((g, w))
                active = keep

        def chains_for(j, es):
            return [('e', j, e) for e in es]

        def vchains_for(j):
            return [('v', j, tt) for tt in range(4)]

        # ================= stage schedule =================
        # S0: first half of qkv(0) -- DMA-paced; warmup matmuls fill the
        # stalls while the first xt/wqk chunks stream in
        g0a = qkv_gen(chains_for(0, [0, 4]))
        drive((g0a, 1), (warm_gen(8), 1))
        g0b = qkv_gen(chains_for(0, [1, 5]) + vchains_for(0), prefetch=1)
        for _ in g0b:
            pass

        # S1: attn(0) x [rest of qkv(0) + qkv(1)]
        g_q1 = qkv_gen(chains_for(0, [2, 6, 3, 7])
                       + chains_for(1, [0, 4, 1, 5, 2, 6, 3, 7])
                       + vchains_for(1),
                       prefetch=2)
        drive((attn_gen(0), 1), (g_q1, 4))

        # S2: attn(1) x qkv(2) x proj(0)
        g_q2 = qkv_gen(chains_for(2, [0, 4, 1, 5, 2, 6, 3, 7])
                       + vchains_for(2),
                       prefetch=3)
        drive((attn_gen(1), 2), (g_q2, 3), (proj_gen(0), 1))

        # S3: attn(2) x qkv(3); proj(1) deferred to the ACT-bound S4
        g_q3 = qkv_gen(chains_for(3, [0, 4, 1, 5, 2, 6, 3, 7])
                       + vchains_for(3))
        drive((attn_gen(2), 1), (g_q3, 1))

        # S4: attn(3) x proj(1) x proj(2) -- attention weighted so it
        # finishes with proj runway left (lets the final norms drain
        # before S5's first tile needs ot(3))
        drive((attn_gen(3), 5), (proj_gen(1), 1),
              (proj_gen(2, pool=psmm, tag="mm"), 1))

        # S5: proj(3) -- per-tile output DMAs to shorten the final drain
        for _ in proj_gen(3, pool=psmm, tag="mm", row_batch=False):
            pass

    nc.compile()
    return nc


def _get_nc():
    if "nc" not in _NC_CACHE:
        _NC_CACHE["nc"] = _build_nc()
    return _NC_CACHE["nc"]


def _host_prep(x, Wqkv, Wproj, mask):
    """Build the 8 per-core input maps (host-side layout transforms)."""
    x = np.asarray(x, dtype=np.float32)
    Wqkv = np.asarray(Wqkv, dtype=np.float32)
    Wproj = np.asarray(Wproj, dtype=np.float32)
    mask = np.asarray(mask, dtype=np.float32)

    # RoPE tables (transposed layout [hd, T]); standard rotate-half RoPE
    # with base 10000.  Signs folded into sin; sinr additionally rolled
    # by 64 partitions so the DVE half-width muls read input and table
    # from the same partition window.
    inv_freq = 1.0 / (10000.0 ** (np.arange(0, HD, 2, dtype=np.float32) / HD))
    freqs = np.arange(T, dtype=np.float32)[:, None] * inv_freq[None, :]
    emb = np.concatenate([freqs, freqs], axis=-1)        # [T, 128]
    cosT = np.ascontiguousarray(np.cos(emb).T).astype(bf16)
    sinT_f = np.ascontiguousarray(np.sin(emb).T)
    sinT_f[:HD // 2] *= -1.0
    sinr = np.ascontiguousarray(np.roll(sinT_f, HD // 2, axis=0)).astype(bf16)

    # [s', t''] triangle for the diagonal 128x128 tile, from the real mask
    tri = (mask[:P, :P].T == 0.0).astype(bf16)
    ones = np.ones((P, P), dtype=bf16)

    in_maps = []
    for b in range(B):
        xT = np.ascontiguousarray(x[b].T).astype(bf16)      # [D, T]
        # [NB, P, NKT, BK]: per (j, partition) rows are 16KB contiguous
        xt_pre = np.ascontiguousarray(
            xT.reshape(NKT, P, NB, BK).transpose(2, 1, 0, 3))
        for g in range(NG):
            heads = list(range(HPG * g, HPG * (g + 1)))
            wq = [Wqkv[:, h * HD:(h + 1) * HD] for h in heads]
            wk = [Wqkv[:, D + h * HD:D + (h + 1) * HD] for h in heads]
            wvl = [Wqkv[:, 2 * D + h * HD:2 * D + (h + 1) * HD] for h in heads]
            # [8, P, NKT, HD]: per-(e,partition) rows 4KB contiguous
            wqk = np.ascontiguousarray(
                np.stack(wq + wk, axis=0).astype(bf16)
                .reshape(2 * HPG, NKT, P, HD).transpose(0, 2, 1, 3))
            wv = np.ascontiguousarray(
                np.concatenate(wvl, axis=1).astype(bf16)
                .reshape(NKT, P, HPG * HD).transpose(1, 0, 2))
            wp = np.ascontiguousarray(
                Wproj[HPG * HD * g:HPG * HD * (g + 1), :].astype(bf16)
                .reshape(HPG, P, D).transpose(1, 0, 2))
            in_maps.append({
                "xt": xt_pre, "wqk": wqk, "wv": wv, "wp": wp,
                "cos": cosT, "sinr": sinr, "tri": tri, "ones": ones,
            })
    return in_maps


def run(x, Wqkv, Wproj, bproj, mask, trace=False):
    """Run the SPMD kernel; returns (output, BassKernelResults)."""
    from concourse.bass_utils import run_bass_kernel_spmd

    nc = _get_nc()
    in_maps = _host_prep(x, Wqkv, Wproj, mask)
    res = run_bass_kernel_spmd(nc, in_maps, core_ids=list(range(B * NG)),
                               trace=trace)

    bproj = np.asarray(bproj, dtype=np.float32)
    out = np.zeros((B, T, D), dtype=np.float32)
    for b in range(B):
        acc = np.zeros((T, D), dtype=np.float32)
        for g in range(NG):
            acc += np.asarray(res.results[b * NG + g]["out"], dtype=np.float32)
        out[b] = acc + bproj[None, :]
    return out, res


def kernel(x, Wqkv, Wproj, bproj, mask):
    out, _ = run(x, Wqkv, Wproj, bproj, mask, trace=False)
    return out
